# revision 18
# baseline (speedup 1.0000x reference)
"""Trainium2 Bass kernel for nn_RealAgnosticInteractionBlock (MACE-style
GNN interaction block): linear_up -> radial MLP -> per-edge CG tensor
product -> scatter-sum -> linear -> skip tensor product -> residual mix.

Strategy (8 NeuronCores, SPMD, no collectives):
  - Host partitions RECEIVER nodes into 80 balanced blocks (FFD bin-packing
    on in-degree), 10 blocks per core; every edge goes to the core owning
    its receiver.  Per-core edge lists are grouped by block and padded to a
    uniform size, so one program serves all 8 cores.
  - Each core: builds the full linear_up'd node table in DRAM (replicated
    compute), gathers sender rows with dma_gather, computes radial-MLP
    tensor-product messages, scatters them into per-block PSUM via one-hot
    matmuls, then applies linear/skip_tp/residual in a transposed node
    stage.  Host unpermutes and reassembles the full output.
"""

import math
import os
from contextlib import ExitStack

import numpy as np

import concourse.bass as bass
import concourse.tile as tile
import concourse.mybir as mybir
import bass_rust
from concourse import bacc
from concourse.alu_op_type import AluOpType
from concourse.vector_clock import ScopedClock

F32 = mybir.dt.float32
F32R = mybir.dt.float32r
BF16 = mybir.dt.bfloat16
I16 = mybir.dt.int16
AX = bass_rust.AxisListType
AF = mybir.ActivationFunctionType

N_NODES = 10000
MUL = 128
N_SPECIES = 4
N_RADIAL = 8
AVG_NEIGH = 16.0
SQRT3 = 1.7320508075688772
N_CORES = 8
NBLK = 10                # node blocks per core
BINS = N_CORES * NBLK    # 80
NODES_PC = NBLK * 128    # 1280 node slots per core
NODE_PAD = BINS * 128    # 10240

MAX_WAITS = 1


class SplitDrainTileContext(tile.TileContext):
    """This compiler build rejects instructions with >1 sem-wait condition
    ("Too many sync wait commands").  Tile's end-of-kernel drain accumulates
    one wait per live sem; split them across preceding SP nops."""

    def _drain_and_barrier(self, tick_clock, wait_clock):
        nc = self.nc
        probe = nc.sync.nop(nofuse=True, hint="drain_wait_split")
        wait_clock.add_sem_waits(
            probe.ins, ScopedClock({None: tick_clock.global_clock})
        )
        si = probe.ins.sync_info
        waits = list(si.on_wait) if si is not None else []
        if si is not None:
            si.on_wait = waits[:MAX_WAITS]
        for i in range(MAX_WAITS, len(waits), MAX_WAITS):
            n2 = nc.sync.nop(nofuse=True, hint=f"drain_wait_split_{i}")
            n2.ins.sync_info = mybir.SyncInfo(
                on_wait=waits[i : i + MAX_WAITS], on_update=[]
            )
        nc.sync.drain()
        nc.all_engine_barrier()
        assert self.sems is not None
        popped = nc._tile_sem_poison_stack.pop()
        assert popped is self._sem_poison
        nc.clear_and_free_semaphores(list(self.sems.allocated().values()))
        nc.all_engine_barrier()


# ---------------------------------------------------------------- host prep


def _ffd_partition(receiver: np.ndarray):
    """LPT-assign nodes to BINS blocks of <=128 nodes balancing in-degree.
    Returns (S, node_bin, node_row) with every block's edge count <= S."""
    deg = np.bincount(receiver, minlength=N_NODES)
    order = np.argsort(-deg, kind="stable")
    load = np.zeros(BINS, dtype=np.int64)
    cnt = np.zeros(BINS, dtype=np.int64)
    node_bin = np.full(N_NODES, -1, dtype=np.int64)
    node_row = np.full(N_NODES, -1, dtype=np.int64)
    big = np.int64(1) << 60
    for n in order:
        masked = np.where(cnt < 128, load, big)
        b = int(np.argmin(masked))
        assert cnt[b] < 128
        node_bin[n] = b
        node_row[n] = cnt[b]
        load[b] += deg[n]
        cnt[b] += 1
    S = max(128, int(-(-load.max() // 128)) * 128)
    return S, node_bin, node_row


def _host_prep(inputs):
    f32 = np.float32
    node_attrs = np.asarray(inputs["node_attrs"], f32)
    node_feats = np.asarray(inputs["node_feats"], f32)
    edge_attrs = np.asarray(inputs["edge_attrs"], f32)
    edge_feats = np.asarray(inputs["edge_feats"], f32)
    edge_index = np.asarray(inputs["edge_index"])
    sender = edge_index[0].astype(np.int64)
    receiver = edge_index[1].astype(np.int64)
    E = sender.shape[0]

    # residual-mix coefficients (scalar host math on a [1] input)
    ucp = float(np.asarray(inputs["update_coeff_param"]).reshape(-1)[0])
    c = 1.0 / (1.0 + math.exp(-ucp))
    c_old = 1.0 / math.sqrt(c * c + 1.0)
    c_new = c * c_old

    # folded weights
    inv_up = 1.0 / math.sqrt(MUL)
    inv_lin = 1.0 / (math.sqrt(2 * MUL) * AVG_NEIGH)
    inv_sk = 1.0 / math.sqrt(MUL * N_SPECIES)
    Wup0 = (np.asarray(inputs["W_up0"], f32) * inv_up)        # [128,128]
    Wup1 = (np.asarray(inputs["W_up1"], f32) * inv_up)
    W1 = np.asarray(inputs["W_mlp1"], f32) / math.sqrt(N_RADIAL)   # [8,64]
    W2 = np.asarray(inputs["W_mlp2"], f32) / math.sqrt(64.0)       # [64,64]
    W3 = (np.asarray(inputs["W_mlp3"], f32) / math.sqrt(64.0)).copy()  # [64,512]
    W3[:, MUL : 2 * MUL] /= SQRT3                              # fold 1/sqrt(3) into w_b
    # stored as [128, n_blocks*128]: block b cols = rows b*128..(b+1)*128
    Wlin0 = np.asarray(inputs["W_lin0"], f32) * inv_lin        # [256,128]
    Wlin0 = np.concatenate([Wlin0[:128], Wlin0[128:]], axis=1)  # [128,256]
    Wlin1 = np.asarray(inputs["W_lin1"], f32) * inv_lin
    Wlin1 = np.concatenate([Wlin1[:128], Wlin1[128:]], axis=1)
    Wsk0 = (
        np.transpose(np.asarray(inputs["W_skip0"], f32), (1, 0, 2)).reshape(512, 128)
        * (inv_sk * c_new)
    )
    Wsk0 = np.concatenate([Wsk0[v * 128 : (v + 1) * 128] for v in range(4)], axis=1)
    Wsk1 = (
        np.transpose(np.asarray(inputs["W_skip1"], f32), (1, 0, 2)).reshape(512, 128)
        * (inv_sk * c_new)
    )
    Wsk1 = np.concatenate([Wsk1[v * 128 : (v + 1) * 128] for v in range(4)], axis=1)
    identC = (np.eye(128, dtype=f32) * c_old).copy()
    ident = np.eye(128, dtype=f32)
    iota = np.broadcast_to(np.arange(128, dtype=f32), (128, 128)).copy()
    ones1 = np.ones((1, 128), f32)

    # transposed, component-split raw node features [512, NODE_PAD] (bf16)
    import ml_dtypes
    bf16 = ml_dtypes.bfloat16
    nfT = np.zeros((512, NODE_PAD), bf16)
    nfT[0:128, :N_NODES] = node_feats[:, :MUL].T
    v = node_feats[:, MUL:].reshape(N_NODES, MUL, 3)
    nfT[128:256, :N_NODES] = v[:, :, 0].T
    nfT[256:384, :N_NODES] = v[:, :, 1].T
    nfT[384:512, :N_NODES] = v[:, :, 2].T

    # graph partition
    S, node_bin, node_row = _ffd_partition(receiver)
    TPB = S // 128                  # subtiles per block
    EP = NBLK * S                   # padded edges per core
    n_sub = EP // 128               # subtiles per core
    chunks = []                     # (edge_offset, n_edges)
    off = 0
    while off < EP:
        g = min(512, EP - off)
        chunks.append((off, g))
        off += g
    NCH = len(chunks)

    # per-core node lists
    core_nodes = np.full((N_CORES, NODES_PC), -1, dtype=np.int64)
    slot_of_node = node_bin * 128 + node_row
    for n in range(N_NODES):
        b = node_bin[n]
        core_nodes[b // NBLK, (b % NBLK) * 128 + node_row[n]] = n

    # per-core edge slots
    e_bin = node_bin[receiver]
    e_core = e_bin // NBLK
    e_blk = e_bin % NBLK
    sender16 = np.zeros((N_CORES, EP), np.int16)
    scal = np.zeros((N_CORES, EP, 8), f32)
    scal[:, :, 4] = -1.0
    efTc = np.zeros((N_CORES, 8, EP), f32)
    for k in range(N_CORES):
        for j in range(NBLK):
            es = np.nonzero((e_core == k) & (e_blk == j))[0]
            L = len(es)
            assert L <= S, (k, j, L, S)
            sl = j * S + np.arange(L)
            sender16[k, sl] = sender[es].astype(np.int16)
            scal[k, sl, 0] = edge_attrs[es, 0]
            scal[k][sl, 1:4] = edge_attrs[es, 1:4]
            scal[k, sl, 4] = node_row[receiver[es]].astype(f32)
            efTc[k][:, sl] = edge_feats[es, :].T

    # pack per-chunk SBUF layouts
    scal2 = np.zeros((N_CORES, NCH, 128, 32), f32)
    idx2 = np.zeros((N_CORES, NCH, 128, 32), np.int16)
    for ci, (o, g) in enumerate(chunks):
        t = g // 128
        blk = scal[:, o : o + g, :].reshape(N_CORES, t, 128, 8)
        scal2[:, ci, :, : t * 8] = np.transpose(blk, (0, 2, 1, 3)).reshape(
            N_CORES, 128, t * 8
        )
        w = sender16[:, o : o + g].reshape(N_CORES, g // 16, 16)
        idx2[:, ci, :16, : g // 16] = np.transpose(w, (0, 2, 1))
        for rep in range(1, 8):
            idx2[:, ci, rep * 16 : (rep + 1) * 16, :] = idx2[:, ci, :16, :]

    # flattened to one partition row: base_partition of matmul rhs must be 0
    aT = np.zeros((N_CORES, 1, 4 * NODES_PC), f32)
    for k in range(N_CORES):
        valid = core_nodes[k] >= 0
        a4 = np.zeros((4, NODES_PC), f32)
        a4[:, valid] = node_attrs[core_nodes[k][valid]].T
        aT[k, 0, :] = a4.reshape(-1)

    shared = dict(
        nfT=nfT, Wup0=Wup0.astype(bf16), Wup1=Wup1.astype(bf16),
        W1=W1.astype(bf16), W2=W2.astype(bf16), W3=W3.astype(bf16),
        Wlin0=Wlin0, Wlin1=Wlin1, Wsk0=Wsk0, Wsk1=Wsk1,
        identC=identC, ident=ident, iota=iota, ones1=ones1,
    )
    in_maps = []
    for k in range(N_CORES):
        m = dict(shared)
        m["scal2"] = scal2[k]
        m["idx2"] = idx2[k]
        m["efT"] = np.ascontiguousarray(efTc[k]).astype(bf16)
        m["aT"] = np.ascontiguousarray(aT[k])
        in_maps.append(m)

    return dict(
        S=S, TPB=TPB, EP=EP, NCH=NCH, chunks=chunks, n_sub=n_sub,
        core_nodes=core_nodes, in_maps=in_maps,
    )


# ------------------------------------------------------------- bass program


def _ap(base, col_off, dims):
    """Sub-AP of a tile AP: keep partition dim, replace free dims."""
    return bass.AP(base.tensor, base.offset + col_off, [base.ap[0]] + dims)


def _build_program(prep):
    S = prep["S"]
    TPB = prep["TPB"]
    EP = prep["EP"]
    chunks = prep["chunks"]
    NCH = prep["NCH"]

    nc = bacc.Bacc("TRN2", target_bir_lowering=False, debug=False)

    din = lambda name, shape, dt=F32: nc.dram_tensor(
        name, shape, dt, kind="ExternalInput"
    ).ap()
    nfT = din("nfT", [512, NODE_PAD], BF16)
    Wup0 = din("Wup0", [128, 128], BF16); Wup1 = din("Wup1", [128, 128], BF16)
    W1 = din("W1", [8, 64], BF16); W2 = din("W2", [64, 64], BF16)
    W3 = din("W3", [64, 512], BF16)
    Wlin0 = din("Wlin0", [128, 256], F32R); Wlin1 = din("Wlin1", [128, 256], F32R)
    Wsk0 = din("Wsk0", [128, 512], F32R); Wsk1 = din("Wsk1", [128, 512], F32R)
    identC = din("identC", [128, 128], F32R); ident = din("ident", [128, 128])
    iota = din("iota", [128, 128]); ones1 = din("ones1", [1, 128], F32R)
    scal2 = din("scal2", [NCH, 128, 32])
    idx2 = din("idx2", [NCH, 128, 32], I16)
    efT = din("efT", [8, EP], BF16)
    aT = din("aT", [1, 4 * NODES_PC], F32R)

    table = nc.dram_tensor("table", [NODE_PAD, 512], BF16, kind="Internal").ap()
    outT = nc.dram_tensor("outT", [512, NODES_PC], F32, kind="ExternalOutput").ap()

    r = lambda ap: ap.bitcast(F32R)

    with SplitDrainTileContext(nc) as tc, ExitStack() as octx:
        wpool = octx.enter_context(tc.tile_pool(name="w", bufs=1))

        def wtile(src, shape, dt=F32):
            t = wpool.tile(shape, dt, tag=src.tensor.name)
            nc.sync.dma_start(t[:], src)
            return t

        wup0 = wtile(Wup0, [128, 128], BF16); wup1 = wtile(Wup1, [128, 128], BF16)
        w1 = wtile(W1, [8, 64], BF16); w2 = wtile(W2, [64, 64], BF16)
        w3 = wtile(W3, [64, 512], BF16)
        wlin = [wtile(Wlin0, [128, 256], F32R), wtile(Wlin1, [128, 256], F32R)]
        wsk = [wtile(Wsk0, [128, 512], F32R), wtile(Wsk1, [128, 512], F32R)]
        idc = wtile(identC, [128, 128], F32R); idn = wtile(ident, [128, 128])
        iot = wtile(iota, [128, 128]); on1 = wtile(ones1, [1, 128], F32R)
        at_sb = wtile(aT, [1, 4 * NODES_PC], F32R)
        msvT = octx.enter_context(tc.tile_pool(name="msvT", bufs=1)).tile(
            [128, 8 * NODES_PC], F32R
        )

        # ---------------- stage A: node table = linear_up(node_feats)
        with ExitStack() as actx:
            apool = actx.enter_context(tc.tile_pool(name="stA", bufs=3))
            apsum = actx.enter_context(
                tc.tile_pool(name="stAp", bufs=2, space="PSUM")
            )
            for t in range(NODE_PAD // 128):
                nf4 = apool.tile([128, 512], BF16, tag="nf4")
                src = bass.AP(
                    nfT.tensor,
                    t * 128,
                    [[NODE_PAD, 128], [128 * NODE_PAD, 4], [1, 128]],
                )
                nc.sync.dma_start(nf4[:].rearrange("p (b j) -> p b j", b=4), src)
                ps = apsum.tile([128, 512], F32, tag="psA")
                for b in range(4):
                    w_b = wup0 if b == 0 else wup1
                    nc.tensor.matmul(
                        ps[:, b * 128 : (b + 1) * 128],
                        nf4[:, b * 128 : (b + 1) * 128],
                        w_b[:],
                        start=True,
                        stop=True,
                    )
                ta = apool.tile([128, 512], BF16, tag="tA")
                nc.scalar.copy(ta[:], ps[:])
                nc.sync.dma_start(table[t * 128 : (t + 1) * 128, :], ta[:])

        # ---------------- stage B: edge pipeline
        with ExitStack() as bctx:
            bpool = bctx.enter_context(tc.tile_pool(name="stB", bufs=2))
            spool = bctx.enter_context(tc.tile_pool(name="stBs", bufs=3))
            ph1 = bctx.enter_context(tc.tile_pool(name="ph1", bufs=1, space="PSUM"))
            ph2 = bctx.enter_context(tc.tile_pool(name="ph2", bufs=1, space="PSUM"))
            ptp = bctx.enter_context(tc.tile_pool(name="ptp", bufs=2, space="PSUM"))
            pbl = bctx.enter_context(tc.tile_pool(name="pbl", bufs=2, space="PSUM"))

            gs = 0  # global subtile index
            blkp = None  # [lo, hi] psum pair
            for ci, (off, G) in enumerate(chunks):
                T = G // 128
                idx_t = bpool.tile([128, 32], I16, tag="idx")
                nc.sync.dma_start(idx_t[:], idx2[ci])
                sc_t = bpool.tile([128, 32], F32, tag="scal")
                nc.sync.dma_start(sc_t[:], scal2[ci])
                ef_t = bpool.tile([8, 512], BF16, tag="ef")
                nc.sync.dma_start(ef_t[:, :G], efT[:, off : off + G])
                g_t = bpool.tile([128, 4, 512], BF16, tag="gath")
                nc.gpsimd.dma_gather(
                    out_ap=g_t[:, :T, :],
                    in_ap=table[:],
                    idxs_ap=idx_t[:],
                    num_idxs=G,
                    num_idxs_reg=G,
                    elem_size=512,
                )
                h1p = ph1.tile([64, 512], F32, tag="h1")
                nc.tensor.matmul(h1p[:, :G], w1[:], ef_t[:, :G], start=True, stop=True)
                h1s = spool.tile([64, 512], BF16, tag="h1s")
                nc.scalar.activation(h1s[:, :G], h1p[:, :G], AF.Silu)
                h2p = ph2.tile([64, 512], F32, tag="h2")
                nc.tensor.matmul(h2p[:, :G], w2[:], h1s[:, :G], start=True, stop=True)
                h2s = spool.tile([64, 512], BF16, tag="h2s")
                nc.scalar.activation(h2s[:, :G], h2p[:, :G], AF.Silu)

                # chunk-level tensors: tps4 [128, 4*512], R [128, 4*1024]
                tps4 = spool.tile([128, 2048], BF16, tag="tps4")
                Rc = spool.tile([128, 4096], BF16, tag="Rc")
                for s in range(T):
                    tpw = ptp.tile([128, 512], F32, tag="tpw")
                    nc.tensor.matmul(
                        tpw[:], h2s[:, s * 128 : (s + 1) * 128], w3[:],
                        start=True, stop=True,
                    )
                    nc.scalar.copy(tps4[:, s * 512 : (s + 1) * 512], tpw[:])
                # R per subtile s: [A|C|Bx|Dx|By|Dy|Bz|Dz] at s*1024
                #   A=wa*xs  C=wc*xs  B_i=wb*xv_i  D_i=wd*xv_i
                nc.vector.tensor_tensor(
                    _ap(Rc[:], 0, [[1024, T], [128, 2], [1, 128]]),
                    _ap(tps4[:], 0, [[512, T], [256, 2], [1, 128]]),
                    _ap(g_t[:], 0, [[512, T], [0, 2], [1, 128]]),
                    AluOpType.mult,
                )
                nc.vector.tensor_tensor(
                    _ap(Rc[:], 256, [[1024, T], [256, 3], [1, 128]]),
                    _ap(tps4[:], 128, [[512, T], [0, 3], [1, 128]]),
                    _ap(g_t[:], 128, [[512, T], [128, 3], [1, 128]]),
                    AluOpType.mult,
                )
                nc.vector.tensor_tensor(
                    _ap(Rc[:], 384, [[1024, T], [256, 3], [1, 128]]),
                    _ap(tps4[:], 384, [[512, T], [0, 3], [1, 128]]),
                    _ap(g_t[:], 128, [[512, T], [128, 3], [1, 128]]),
                    AluOpType.mult,
                )

                for s in range(T):
                    j = gs // TPB      # node block
                    sin = gs % TPB     # subtile within block
                    gs += 1
                    first = sin == 0
                    last = sin == TPB - 1

                    # oh4 = [oh*ea_s | oh*eav_x | oh*eav_y | oh*eav_z]
                    rrel = sc_t[:, s * 8 + 4 : s * 8 + 5]
                    oh4 = spool.tile([128, 512], BF16, tag="oh4")
                    nc.vector.scalar_tensor_tensor(
                        oh4[:],
                        _ap(iot[:], 0, [[0, 4], [1, 128]]),
                        rrel,
                        _ap(sc_t[:], s * 8, [[1, 4], [0, 128]]),
                        AluOpType.is_equal,
                        AluOpType.mult,
                    )
                    if first:
                        blkp = [
                            pbl.tile([128, 512], F32, tag="blkA", name=f"blkA_{j}"),
                            pbl.tile([128, 512], F32, tag="blkB", name=f"blkB_{j}"),
                        ]
                        openers = [None, None]
                    # psum: blkA=[m0|m1|m2x|m2y] blkB=[m2z|m3x|m3y|m3z]
                    oh_s = oh4[:, 0:128]
                    Rb = s * 1024

                    def smm(bank, dst, lhsT, rhs, stop=False):
                        mm = nc.tensor.matmul(
                            dst, lhsT, rhs,
                            start=(openers[bank] is None), stop=stop,
                        )
                        if openers[bank] is None:
                            openers[bank] = mm.ins
                        else:
                            tile.add_dep_helper(
                                mm.ins, openers[bank], sync=False,
                                reason="psum bank zero-order",
                            )
                        return mm

                    smm(0, blkp[0][:, 0:128], oh_s, Rc[:, Rb : Rb + 128])
                    smm(1, blkp[1][:, 0:128], oh4[:, 384:512],
                        Rc[:, Rb + 128 : Rb + 256])  # m2z
                    smm(
                        1,
                        blkp[1][:, 128:512],
                        oh_s,
                        _ap(Rc[:], Rb + 384, [[256, 3], [1, 128]]),
                        stop=last,
                    )
                    for i3 in range(3):
                        oh_i = oh4[:, (1 + i3) * 128 : (2 + i3) * 128]
                        if i3 < 2:
                            smm(0, blkp[0][:, (2 + i3) * 128 : (3 + i3) * 128],
                                oh_i, Rc[:, Rb + 128 : Rb + 256])
                        smm(
                            0,
                            blkp[0][:, 128:256],
                            oh_i,
                            Rc[:, Rb + 256 + 256 * i3 : Rb + 384 + 256 * i3],
                            stop=(last and i3 == 2),
                        )
                    if last:
                        # msv = [m0|m1|m2x|m2y|m2z|m3x|m3y|m3z]
                        msv = spool.tile([128, 1024], F32, tag="msv")
                        nc.vector.tensor_copy(msv[:, 0:512], blkp[0][:])
                        nc.vector.tensor_copy(msv[:, 512:1024], blkp[1][:])
                        srcs = [
                            msv[:, 0:128],      # m0
                            msv[:, 128:256],    # m1
                            msv[:, 256:384],    # m2x
                            msv[:, 640:768],    # m3x
                            msv[:, 384:512],    # m2y
                            msv[:, 768:896],    # m3y
                            msv[:, 512:640],    # m2z
                            msv[:, 896:1024],   # m3z
                        ]
                        for j8, sap in enumerate(srcs):
                            trp = ptp.tile([128, 128], F32, tag="tpw",
                                           name=f"trp_{j}_{j8}")
                            nc.tensor.transpose(trp[:], sap, idn[:])
                            nc.scalar.copy(
                                msvT[:, j8 * NODES_PC + j * 128 : j8 * NODES_PC + (j + 1) * 128],
                                trp[:],
                            )

        # ---------------- stage C: node stage (transposed layout)
        with ExitStack() as cctx:
            cpool = cctx.enter_context(tc.tile_pool(name="stC", bufs=2))
            arpool = cctx.enter_context(tc.tile_pool(name="stCar", bufs=4))
            pcs = cctx.enter_context(tc.tile_pool(name="pcs", bufs=2, space="PSUM"))
            pco = cctx.enter_context(tc.tile_pool(name="pco", bufs=2, space="PSUM"))
            pca = cctx.enter_context(tc.tile_pool(name="pca", bufs=2, space="PSUM"))

            n0 = 0
            while n0 < NODES_PC:
                W = min(512, NODES_PC - n0)
                arep = []
                for vv in range(4):
                    pa = pca.tile([128, 512], F32, tag="pa")
                    nc.tensor.matmul(
                        pa[:, :W],
                        r(on1[:]),
                        r(at_sb[0:1, vv * NODES_PC + n0 : vv * NODES_PC + n0 + W]),
                        start=True, stop=True,
                    )
                    av = arpool.tile([128, 512], F32, tag=f"arep{vv}")
                    nc.vector.tensor_copy(av[:, :W], pa[:, :W])
                    arep.append(av)
                for comp in range(4):
                    wl = wlin[0] if comp == 0 else wlin[1]
                    wsx = wsk[0] if comp == 0 else wsk[1]
                    psS = pcs.tile([128, 512], F32, tag="psS")
                    for half in range(2):
                        jt = 2 * comp + half
                        nc.tensor.matmul(
                            psS[:, :W],
                            r(wl[:, half * 128 : (half + 1) * 128]),
                            r(msvT[:, jt * NODES_PC + n0 : jt * NODES_PC + n0 + W]),
                            start=(half == 0),
                            stop=(half == 1),
                        )
                    ssb = cpool.tile([128, 512], F32R, tag="ssb")
                    nc.vector.tensor_copy(ssb[:, :W], psS[:, :W])
                    psO = pco.tile([128, 512], F32, tag="psO")
                    nc.tensor.matmul(
                        psO[:, :W], r(idc[:]), r(ssb[:, :W]), start=True, stop=False
                    )
                    for vv in range(4):
                        pv = cpool.tile([128, 512], F32R, tag="pv")
                        nc.vector.tensor_tensor(
                            pv[:, :W], ssb[:, :W], arep[vv][:, :W], AluOpType.mult
                        )
                        nc.tensor.matmul(
                            psO[:, :W],
                            r(wsx[:, vv * 128 : (vv + 1) * 128]),
                            r(pv[:, :W]),
                            start=False,
                            stop=(vv == 3),
                        )
                    osb = cpool.tile([128, 512], F32, tag="osb")
                    nc.vector.tensor_copy(osb[:, :W], psO[:, :W])
                    nc.sync.dma_start(
                        outT[comp * 128 : (comp + 1) * 128, n0 : n0 + W], osb[:, :W]
                    )
                n0 += W

    nc.compile()
    return nc


# ---------------------------------------------------------------- top level


def _unshard(results, prep):
    core_nodes = prep["core_nodes"]
    out = np.zeros((N_NODES, MUL, 4), np.float32)
    for k in range(N_CORES):
        oT = results[k]["outT"]  # [512, 1280]
        valid = core_nodes[k] >= 0
        nodes = core_nodes[k][valid]
        cols = np.nonzero(valid)[0]
        for comp in range(4):
            out[nodes, :, comp] = oT[comp * 128 : (comp + 1) * 128, cols].T
    return out


def kernel(**inputs):
    from concourse import bass_utils

    prep = _host_prep(inputs)
    nc = _build_program(prep)
    trace = bool(int(os.environ.get("KERNEL_TRACE", "0")))
    res = bass_utils.run_bass_kernel_spmd(
        nc,
        prep["in_maps"],
        core_ids=list(range(N_CORES)),
        trace=trace,
    )
    if trace:
        kernel.last_results = res
    return _unshard(res.results, prep)


kernel.last_results = None


# revision 19
# speedup vs baseline: 1.0824x; 1.0824x over previous
"""Trainium2 Bass kernel for nn_RealAgnosticInteractionBlock (MACE-style
GNN interaction block): linear_up -> radial MLP -> per-edge CG tensor
product -> scatter-sum -> linear -> skip tensor product -> residual mix.

Strategy (8 NeuronCores, SPMD, no collectives):
  - Host partitions RECEIVER nodes into 80 balanced blocks (FFD bin-packing
    on in-degree), 10 blocks per core; every edge goes to the core owning
    its receiver.  Per-core edge lists are grouped by block and padded to a
    uniform size, so one program serves all 8 cores.
  - Each core: builds the full linear_up'd node table in DRAM (replicated
    compute), gathers sender rows with dma_gather, computes radial-MLP
    tensor-product messages, scatters them into per-block PSUM via one-hot
    matmuls, then applies linear/skip_tp/residual in a transposed node
    stage.  Host unpermutes and reassembles the full output.
"""

import math
import os
from contextlib import ExitStack

import numpy as np

import concourse.bass as bass
import concourse.tile as tile
import concourse.mybir as mybir
import bass_rust
from concourse import bacc
from concourse.alu_op_type import AluOpType
from concourse.vector_clock import ScopedClock

F32 = mybir.dt.float32
F32R = mybir.dt.float32r
BF16 = mybir.dt.bfloat16
I16 = mybir.dt.int16
AX = bass_rust.AxisListType
AF = mybir.ActivationFunctionType

N_NODES = 10000
MUL = 128
N_SPECIES = 4
N_RADIAL = 8
AVG_NEIGH = 16.0
SQRT3 = 1.7320508075688772
N_CORES = 8
NBLK = 10                # node blocks per core
BINS = N_CORES * NBLK    # 80
NODES_PC = NBLK * 128    # 1280 node slots per core
NODE_PAD = BINS * 128    # 10240

MAX_WAITS = 1


class SplitDrainTileContext(tile.TileContext):
    """This compiler build rejects instructions with >1 sem-wait condition
    ("Too many sync wait commands").  Tile's end-of-kernel drain accumulates
    one wait per live sem; split them across preceding SP nops."""

    def _drain_and_barrier(self, tick_clock, wait_clock):
        nc = self.nc
        probe = nc.sync.nop(nofuse=True, hint="drain_wait_split")
        wait_clock.add_sem_waits(
            probe.ins, ScopedClock({None: tick_clock.global_clock})
        )
        si = probe.ins.sync_info
        waits = list(si.on_wait) if si is not None else []
        if si is not None:
            si.on_wait = waits[:MAX_WAITS]
        for i in range(MAX_WAITS, len(waits), MAX_WAITS):
            n2 = nc.sync.nop(nofuse=True, hint=f"drain_wait_split_{i}")
            n2.ins.sync_info = mybir.SyncInfo(
                on_wait=waits[i : i + MAX_WAITS], on_update=[]
            )
        nc.sync.drain()
        nc.all_engine_barrier()
        assert self.sems is not None
        popped = nc._tile_sem_poison_stack.pop()
        assert popped is self._sem_poison
        nc.clear_and_free_semaphores(list(self.sems.allocated().values()))
        nc.all_engine_barrier()


# ---------------------------------------------------------------- host prep


def _ffd_partition(receiver: np.ndarray):
    """LPT-assign nodes to BINS blocks of <=128 nodes balancing in-degree.
    Returns (S, node_bin, node_row) with every block's edge count <= S."""
    deg = np.bincount(receiver, minlength=N_NODES)
    order = np.argsort(-deg, kind="stable")
    load = np.zeros(BINS, dtype=np.int64)
    cnt = np.zeros(BINS, dtype=np.int64)
    node_bin = np.full(N_NODES, -1, dtype=np.int64)
    node_row = np.full(N_NODES, -1, dtype=np.int64)
    big = np.int64(1) << 60
    for n in order:
        masked = np.where(cnt < 128, load, big)
        b = int(np.argmin(masked))
        assert cnt[b] < 128
        node_bin[n] = b
        node_row[n] = cnt[b]
        load[b] += deg[n]
        cnt[b] += 1
    S = max(128, int(-(-load.max() // 128)) * 128)
    return S, node_bin, node_row


def _host_prep(inputs):
    f32 = np.float32
    node_attrs = np.asarray(inputs["node_attrs"], f32)
    node_feats = np.asarray(inputs["node_feats"], f32)
    edge_attrs = np.asarray(inputs["edge_attrs"], f32)
    edge_feats = np.asarray(inputs["edge_feats"], f32)
    edge_index = np.asarray(inputs["edge_index"])
    sender = edge_index[0].astype(np.int64)
    receiver = edge_index[1].astype(np.int64)
    E = sender.shape[0]

    # residual-mix coefficients (scalar host math on a [1] input)
    ucp = float(np.asarray(inputs["update_coeff_param"]).reshape(-1)[0])
    c = 1.0 / (1.0 + math.exp(-ucp))
    c_old = 1.0 / math.sqrt(c * c + 1.0)
    c_new = c * c_old

    # folded weights
    inv_up = 1.0 / math.sqrt(MUL)
    inv_lin = 1.0 / (math.sqrt(2 * MUL) * AVG_NEIGH)
    inv_sk = 1.0 / math.sqrt(MUL * N_SPECIES)
    Wup0 = (np.asarray(inputs["W_up0"], f32) * inv_up)        # [128,128]
    Wup1 = (np.asarray(inputs["W_up1"], f32) * inv_up)
    W1 = np.asarray(inputs["W_mlp1"], f32) / math.sqrt(N_RADIAL)   # [8,64]
    W2 = np.asarray(inputs["W_mlp2"], f32) / math.sqrt(64.0)       # [64,64]
    W3 = (np.asarray(inputs["W_mlp3"], f32) / math.sqrt(64.0)).copy()  # [64,512]
    W3[:, MUL : 2 * MUL] /= SQRT3                              # fold 1/sqrt(3) into w_b
    # stored as [128, n_blocks*128]: block b cols = rows b*128..(b+1)*128
    Wlin0 = np.asarray(inputs["W_lin0"], f32) * inv_lin        # [256,128]
    Wlin0 = np.concatenate([Wlin0[:128], Wlin0[128:]], axis=1)  # [128,256]
    Wlin1 = np.asarray(inputs["W_lin1"], f32) * inv_lin
    Wlin1 = np.concatenate([Wlin1[:128], Wlin1[128:]], axis=1)
    Wsk0 = (
        np.transpose(np.asarray(inputs["W_skip0"], f32), (1, 0, 2)).reshape(512, 128)
        * (inv_sk * c_new)
    )
    Wsk0 = np.concatenate([Wsk0[v * 128 : (v + 1) * 128] for v in range(4)], axis=1)
    Wsk1 = (
        np.transpose(np.asarray(inputs["W_skip1"], f32), (1, 0, 2)).reshape(512, 128)
        * (inv_sk * c_new)
    )
    Wsk1 = np.concatenate([Wsk1[v * 128 : (v + 1) * 128] for v in range(4)], axis=1)
    identC = (np.eye(128, dtype=f32) * c_old).copy()
    ident = np.eye(128, dtype=f32)
    iota = np.broadcast_to(np.arange(128, dtype=f32), (128, 128)).copy()
    ones1 = np.ones((1, 128), f32)

    # transposed, component-split raw node features [512, NODE_PAD] (bf16)
    import ml_dtypes
    bf16 = ml_dtypes.bfloat16
    nfT = np.zeros((512, NODE_PAD), bf16)
    nfT[0:128, :N_NODES] = node_feats[:, :MUL].T
    v = node_feats[:, MUL:].reshape(N_NODES, MUL, 3)
    nfT[128:256, :N_NODES] = v[:, :, 0].T
    nfT[256:384, :N_NODES] = v[:, :, 1].T
    nfT[384:512, :N_NODES] = v[:, :, 2].T

    # graph partition
    S, node_bin, node_row = _ffd_partition(receiver)
    TPB = S // 128                  # subtiles per block
    EP = NBLK * S                   # padded edges per core
    n_sub = EP // 128               # subtiles per core
    chunks = []                     # (edge_offset, n_edges)
    off = 0
    while off < EP:
        g = min(512, EP - off)
        chunks.append((off, g))
        off += g
    NCH = len(chunks)

    # per-core node lists
    core_nodes = np.full((N_CORES, NODES_PC), -1, dtype=np.int64)
    slot_of_node = node_bin * 128 + node_row
    for n in range(N_NODES):
        b = node_bin[n]
        core_nodes[b // NBLK, (b % NBLK) * 128 + node_row[n]] = n

    # per-core edge slots
    e_bin = node_bin[receiver]
    e_core = e_bin // NBLK
    e_blk = e_bin % NBLK
    sender16 = np.zeros((N_CORES, EP), np.int16)
    scal = np.zeros((N_CORES, EP, 8), f32)
    scal[:, :, 4] = -1.0
    efTc = np.zeros((N_CORES, 8, EP), f32)
    for k in range(N_CORES):
        for j in range(NBLK):
            es = np.nonzero((e_core == k) & (e_blk == j))[0]
            L = len(es)
            assert L <= S, (k, j, L, S)
            sl = j * S + np.arange(L)
            sender16[k, sl] = sender[es].astype(np.int16)
            scal[k, sl, 0] = edge_attrs[es, 0]
            scal[k][sl, 1:4] = edge_attrs[es, 1:4]
            scal[k, sl, 4] = node_row[receiver[es]].astype(f32)
            efTc[k][:, sl] = edge_feats[es, :].T

    # pack per-chunk SBUF layouts
    scal2 = np.zeros((N_CORES, NCH, 128, 32), f32)
    idx2 = np.zeros((N_CORES, NCH, 128, 32), np.int16)
    for ci, (o, g) in enumerate(chunks):
        t = g // 128
        blk = scal[:, o : o + g, :].reshape(N_CORES, t, 128, 8)
        scal2[:, ci, :, : t * 8] = np.transpose(blk, (0, 2, 1, 3)).reshape(
            N_CORES, 128, t * 8
        )
        w = sender16[:, o : o + g].reshape(N_CORES, g // 16, 16)
        idx2[:, ci, :16, : g // 16] = np.transpose(w, (0, 2, 1))
        for rep in range(1, 8):
            idx2[:, ci, rep * 16 : (rep + 1) * 16, :] = idx2[:, ci, :16, :]

    # flattened to one partition row: base_partition of matmul rhs must be 0
    aT = np.zeros((N_CORES, 1, 4 * NODES_PC), f32)
    for k in range(N_CORES):
        valid = core_nodes[k] >= 0
        a4 = np.zeros((4, NODES_PC), f32)
        a4[:, valid] = node_attrs[core_nodes[k][valid]].T
        aT[k, 0, :] = a4.reshape(-1)

    shared = dict(
        nfT=nfT, Wup0=Wup0.astype(bf16), Wup1=Wup1.astype(bf16),
        W1=W1.astype(bf16), W2=W2.astype(bf16), W3=W3.astype(bf16),
        Wlin0=Wlin0, Wlin1=Wlin1, Wsk0=Wsk0, Wsk1=Wsk1,
        identC=identC, ident=ident, iota=iota, ones1=ones1,
    )
    in_maps = []
    for k in range(N_CORES):
        m = dict(shared)
        m["scal2"] = scal2[k]
        m["idx2"] = idx2[k]
        m["efT"] = np.ascontiguousarray(efTc[k]).astype(bf16)
        m["aT"] = np.ascontiguousarray(aT[k])
        in_maps.append(m)

    return dict(
        S=S, TPB=TPB, EP=EP, NCH=NCH, chunks=chunks, n_sub=n_sub,
        core_nodes=core_nodes, in_maps=in_maps,
    )


# ------------------------------------------------------------- bass program


def _ap(base, col_off, dims):
    """Sub-AP of a tile AP: keep partition dim, replace free dims."""
    return bass.AP(base.tensor, base.offset + col_off, [base.ap[0]] + dims)


def _build_program(prep):
    S = prep["S"]
    TPB = prep["TPB"]
    EP = prep["EP"]
    chunks = prep["chunks"]
    NCH = prep["NCH"]

    nc = bacc.Bacc("TRN2", target_bir_lowering=False, debug=False)

    din = lambda name, shape, dt=F32: nc.dram_tensor(
        name, shape, dt, kind="ExternalInput"
    ).ap()
    nfT = din("nfT", [512, NODE_PAD], BF16)
    Wup0 = din("Wup0", [128, 128], BF16); Wup1 = din("Wup1", [128, 128], BF16)
    W1 = din("W1", [8, 64], BF16); W2 = din("W2", [64, 64], BF16)
    W3 = din("W3", [64, 512], BF16)
    Wlin0 = din("Wlin0", [128, 256], F32R); Wlin1 = din("Wlin1", [128, 256], F32R)
    Wsk0 = din("Wsk0", [128, 512], F32R); Wsk1 = din("Wsk1", [128, 512], F32R)
    identC = din("identC", [128, 128], F32R); ident = din("ident", [128, 128])
    iota = din("iota", [128, 128]); ones1 = din("ones1", [1, 128], F32R)
    scal2 = din("scal2", [NCH, 128, 32])
    idx2 = din("idx2", [NCH, 128, 32], I16)
    efT = din("efT", [8, EP], BF16)
    aT = din("aT", [1, 4 * NODES_PC], F32R)

    table = nc.dram_tensor("table", [NODE_PAD, 512], BF16, kind="Internal").ap()
    outT = nc.dram_tensor("outT", [512, NODES_PC], F32, kind="ExternalOutput").ap()

    r = lambda ap: ap.bitcast(F32R)

    with SplitDrainTileContext(nc) as tc, ExitStack() as octx:
        wpool = octx.enter_context(tc.tile_pool(name="w", bufs=1))

        def wtile(src, shape, dt=F32):
            t = wpool.tile(shape, dt, tag=src.tensor.name)
            nc.sync.dma_start(t[:], src)
            return t

        wup0 = wtile(Wup0, [128, 128], BF16); wup1 = wtile(Wup1, [128, 128], BF16)
        w1 = wtile(W1, [8, 64], BF16); w2 = wtile(W2, [64, 64], BF16)
        w3 = wtile(W3, [64, 512], BF16)
        wlin = [wtile(Wlin0, [128, 256], F32R), wtile(Wlin1, [128, 256], F32R)]
        wsk = [wtile(Wsk0, [128, 512], F32R), wtile(Wsk1, [128, 512], F32R)]
        idc = wtile(identC, [128, 128], F32R); idn = wtile(ident, [128, 128])
        iot = wtile(iota, [128, 128]); on1 = wtile(ones1, [1, 128], F32R)
        at_sb = wtile(aT, [1, 4 * NODES_PC], F32R)
        msvT = octx.enter_context(tc.tile_pool(name="msvT", bufs=1)).tile(
            [128, 8 * NODES_PC], F32R
        )

        # ---------------- stage A: node table = linear_up(node_feats)
        with ExitStack() as actx:
            apool = actx.enter_context(tc.tile_pool(name="stA", bufs=3))
            apsum = actx.enter_context(
                tc.tile_pool(name="stAp", bufs=2, space="PSUM")
            )
            for t in range(NODE_PAD // 128):
                nf4 = apool.tile([128, 512], BF16, tag="nf4")
                src = bass.AP(
                    nfT.tensor,
                    t * 128,
                    [[NODE_PAD, 128], [128 * NODE_PAD, 4], [1, 128]],
                )
                nc.sync.dma_start(nf4[:].rearrange("p (b j) -> p b j", b=4), src)
                ps = apsum.tile([128, 512], F32, tag="psA")
                for b in range(4):
                    w_b = wup0 if b == 0 else wup1
                    nc.tensor.matmul(
                        ps[:, b * 128 : (b + 1) * 128],
                        nf4[:, b * 128 : (b + 1) * 128],
                        w_b[:],
                        start=True,
                        stop=True,
                    )
                ta = apool.tile([128, 512], BF16, tag="tA")
                nc.scalar.copy(ta[:], ps[:])
                nc.sync.dma_start(table[t * 128 : (t + 1) * 128, :], ta[:])

        # ---------------- stage B: edge pipeline
        with ExitStack() as bctx:
            bpool = bctx.enter_context(tc.tile_pool(name="stB", bufs=2))
            spool = bctx.enter_context(tc.tile_pool(name="stBs", bufs=3))
            ph1 = bctx.enter_context(tc.tile_pool(name="ph1", bufs=1, space="PSUM"))
            ph2 = bctx.enter_context(tc.tile_pool(name="ph2", bufs=1, space="PSUM"))
            ptp = bctx.enter_context(tc.tile_pool(name="ptp", bufs=2, space="PSUM"))
            pbl = bctx.enter_context(tc.tile_pool(name="pbl", bufs=1, space="PSUM"))
            ptr = bctx.enter_context(tc.tile_pool(name="ptr", bufs=2, space="PSUM"))

            gs = 0  # global subtile index
            blkp = None  # [lo, hi] psum pair
            for ci, (off, G) in enumerate(chunks):
                T = G // 128
                idx_t = bpool.tile([128, 32], I16, tag="idx")
                nc.sync.dma_start(idx_t[:], idx2[ci])
                sc_t = bpool.tile([128, 32], F32, tag="scal")
                nc.sync.dma_start(sc_t[:], scal2[ci])
                ef_t = bpool.tile([8, 512], BF16, tag="ef")
                nc.sync.dma_start(ef_t[:, :G], efT[:, off : off + G])
                g_t = bpool.tile([128, 4, 512], BF16, tag="gath")
                nc.gpsimd.dma_gather(
                    out_ap=g_t[:, :T, :],
                    in_ap=table[:],
                    idxs_ap=idx_t[:],
                    num_idxs=G,
                    num_idxs_reg=G,
                    elem_size=512,
                )
                h1p = ph1.tile([64, 512], F32, tag="h1")
                nc.tensor.matmul(h1p[:, :G], w1[:], ef_t[:, :G], start=True, stop=True)
                h1s = spool.tile([64, 512], BF16, tag="h1s")
                nc.scalar.activation(h1s[:, :G], h1p[:, :G], AF.Silu)
                h2p = ph2.tile([64, 512], F32, tag="h2")
                nc.tensor.matmul(h2p[:, :G], w2[:], h1s[:, :G], start=True, stop=True)
                h2s = spool.tile([64, 512], BF16, tag="h2s")
                nc.scalar.activation(h2s[:, :G], h2p[:, :G], AF.Silu)

                for s in range(T):
                    j = gs // TPB      # node block
                    sin = gs % TPB     # subtile within block
                    gs += 1
                    first = sin == 0
                    last = sin == TPB - 1

                    tpw = ptp.tile([128, 512], F32, tag="tpw")
                    nc.tensor.matmul(
                        tpw[:], h2s[:, s * 128 : (s + 1) * 128], w3[:],
                        start=True, stop=True,
                    )
                    tps = spool.tile([128, 512], BF16, tag="tps")
                    nc.scalar.copy(tps[:], tpw[:])

                    # oh4 = [oh*ea_s | oh*eav_x | oh*eav_y | oh*eav_z]
                    rrel = sc_t[:, s * 8 + 4 : s * 8 + 5]
                    oh4 = spool.tile([128, 512], BF16, tag="oh4")
                    nc.vector.scalar_tensor_tensor(
                        oh4[:],
                        _ap(iot[:], 0, [[0, 4], [1, 128]]),
                        rrel,
                        _ap(sc_t[:], s * 8, [[1, 4], [0, 128]]),
                        AluOpType.is_equal,
                        AluOpType.mult,
                    )
                    # R = [A|C|Bx|Dx|By|Dy|Bz|Dz]; A=wa*xs C=wc*xs Bi=wb*xv_i Di=wd*xv_i
                    R = spool.tile([128, 1024], BF16, tag="R")
                    nc.vector.tensor_tensor(
                        R[:, 0:256],
                        _ap(tps[:], 0, [[256, 2], [1, 128]]),
                        _ap(g_t[:], s * 512, [[0, 2], [1, 128]]),
                        AluOpType.mult,
                    )
                    nc.vector.tensor_tensor(
                        R[:, 256:1024],
                        _ap(tps[:], 128, [[0, 3], [256, 2], [1, 128]]),
                        _ap(g_t[:], s * 512 + 128, [[128, 3], [0, 2], [1, 128]]),
                        AluOpType.mult,
                    )
                    if first:
                        blkp = [
                            pbl.tile([128, 512], F32, tag="blkA", name=f"blkA_{j}"),
                            pbl.tile([128, 512], F32, tag="blkB", name=f"blkB_{j}"),
                        ]
                        openers = [None, None]
                    # psum: blkA=[m0|m1|m2x|m2y] blkB=[m2z|m3x|m3y|m3z]
                    oh_s = oh4[:, 0:128]

                    def smm(bank, dst, lhsT, rhs, stop=False):
                        mm = nc.tensor.matmul(
                            dst, lhsT, rhs,
                            start=(openers[bank] is None), stop=stop,
                        )
                        if openers[bank] is None:
                            openers[bank] = mm.ins
                        else:
                            tile.add_dep_helper(
                                mm.ins, openers[bank], sync=False,
                                reason="psum bank zero-order",
                            )
                        return mm

                    smm(0, blkp[0][:, 0:128], oh_s, R[:, 0:128])
                    smm(1, blkp[1][:, 0:128], oh4[:, 384:512], R[:, 128:256])  # m2z
                    smm(
                        1,
                        blkp[1][:, 128:512],
                        oh_s,
                        _ap(R[:], 384, [[256, 3], [1, 128]]),
                        stop=last,
                    )
                    for i3 in range(3):
                        oh_i = oh4[:, (1 + i3) * 128 : (2 + i3) * 128]
                        if i3 < 2:
                            smm(0, blkp[0][:, (2 + i3) * 128 : (3 + i3) * 128],
                                oh_i, R[:, 128:256])
                        smm(
                            0,
                            blkp[0][:, 128:256],
                            oh_i,
                            R[:, (256 + 256 * i3) : (384 + 256 * i3)],
                            stop=(last and i3 == 2),
                        )
                    if last:
                        # msv = [m0|m1|m2x|m2y|m2z|m3x|m3y|m3z]
                        msv = spool.tile([128, 1024], F32, tag="msv")
                        nc.vector.tensor_copy(msv[:, 0:512], blkp[0][:])
                        nc.vector.tensor_copy(msv[:, 512:1024], blkp[1][:])
                        srcs = [
                            msv[:, 0:128],      # m0
                            msv[:, 128:256],    # m1
                            msv[:, 256:384],    # m2x
                            msv[:, 640:768],    # m3x
                            msv[:, 384:512],    # m2y
                            msv[:, 768:896],    # m3y
                            msv[:, 512:640],    # m2z
                            msv[:, 896:1024],   # m3z
                        ]
                        for j8, sap in enumerate(srcs):
                            trp = ptr.tile([128, 128], F32, tag="trp")
                            nc.tensor.transpose(trp[:], sap, idn[:])
                            nc.scalar.copy(
                                msvT[:, j8 * NODES_PC + j * 128 : j8 * NODES_PC + (j + 1) * 128],
                                trp[:],
                            )

        # ---------------- stage C: node stage (transposed layout)
        with ExitStack() as cctx:
            cpool = cctx.enter_context(tc.tile_pool(name="stC", bufs=2))
            arpool = cctx.enter_context(tc.tile_pool(name="stCar", bufs=4))
            pcs = cctx.enter_context(tc.tile_pool(name="pcs", bufs=2, space="PSUM"))
            pco = cctx.enter_context(tc.tile_pool(name="pco", bufs=2, space="PSUM"))
            pca = cctx.enter_context(tc.tile_pool(name="pca", bufs=2, space="PSUM"))

            n0 = 0
            while n0 < NODES_PC:
                W = min(512, NODES_PC - n0)
                arep = []
                for vv in range(4):
                    pa = pca.tile([128, 512], F32, tag="pa")
                    nc.tensor.matmul(
                        pa[:, :W],
                        r(on1[:]),
                        r(at_sb[0:1, vv * NODES_PC + n0 : vv * NODES_PC + n0 + W]),
                        start=True, stop=True,
                    )
                    av = arpool.tile([128, 512], F32, tag=f"arep{vv}")
                    nc.vector.tensor_copy(av[:, :W], pa[:, :W])
                    arep.append(av)
                for comp in range(4):
                    wl = wlin[0] if comp == 0 else wlin[1]
                    wsx = wsk[0] if comp == 0 else wsk[1]
                    psS = pcs.tile([128, 512], F32, tag="psS")
                    for half in range(2):
                        jt = 2 * comp + half
                        nc.tensor.matmul(
                            psS[:, :W],
                            r(wl[:, half * 128 : (half + 1) * 128]),
                            r(msvT[:, jt * NODES_PC + n0 : jt * NODES_PC + n0 + W]),
                            start=(half == 0),
                            stop=(half == 1),
                        )
                    ssb = cpool.tile([128, 512], F32R, tag="ssb")
                    nc.vector.tensor_copy(ssb[:, :W], psS[:, :W])
                    psO = pco.tile([128, 512], F32, tag="psO")
                    nc.tensor.matmul(
                        psO[:, :W], r(idc[:]), r(ssb[:, :W]), start=True, stop=False
                    )
                    for vv in range(4):
                        pv = cpool.tile([128, 512], F32R, tag="pv")
                        nc.vector.tensor_tensor(
                            pv[:, :W], ssb[:, :W], arep[vv][:, :W], AluOpType.mult
                        )
                        nc.tensor.matmul(
                            psO[:, :W],
                            r(wsx[:, vv * 128 : (vv + 1) * 128]),
                            r(pv[:, :W]),
                            start=False,
                            stop=(vv == 3),
                        )
                    osb = cpool.tile([128, 512], F32, tag="osb")
                    nc.vector.tensor_copy(osb[:, :W], psO[:, :W])
                    nc.sync.dma_start(
                        outT[comp * 128 : (comp + 1) * 128, n0 : n0 + W], osb[:, :W]
                    )
                n0 += W

    nc.compile()
    return nc


# ---------------------------------------------------------------- top level


def _unshard(results, prep):
    core_nodes = prep["core_nodes"]
    out = np.zeros((N_NODES, MUL, 4), np.float32)
    for k in range(N_CORES):
        oT = results[k]["outT"]  # [512, 1280]
        valid = core_nodes[k] >= 0
        nodes = core_nodes[k][valid]
        cols = np.nonzero(valid)[0]
        for comp in range(4):
            out[nodes, :, comp] = oT[comp * 128 : (comp + 1) * 128, cols].T
    return out


def kernel(**inputs):
    from concourse import bass_utils

    prep = _host_prep(inputs)
    nc = _build_program(prep)
    trace = bool(int(os.environ.get("KERNEL_TRACE", "0")))
    res = bass_utils.run_bass_kernel_spmd(
        nc,
        prep["in_maps"],
        core_ids=list(range(N_CORES)),
        trace=trace,
    )
    if trace:
        kernel.last_results = res
    return _unshard(res.results, prep)


kernel.last_results = None


# revision 20
# speedup vs baseline: 1.1934x; 1.1026x over previous
"""Trainium2 Bass kernel for nn_RealAgnosticInteractionBlock (MACE-style
GNN interaction block): linear_up -> radial MLP -> per-edge CG tensor
product -> scatter-sum -> linear -> skip tensor product -> residual mix.

Strategy (8 NeuronCores, SPMD, no collectives):
  - Host partitions RECEIVER nodes into 80 balanced blocks (FFD bin-packing
    on in-degree), 10 blocks per core; every edge goes to the core owning
    its receiver.  Per-core edge lists are grouped by block and padded to a
    uniform size, so one program serves all 8 cores.
  - Each core: builds the full linear_up'd node table in DRAM (replicated
    compute), gathers sender rows with dma_gather, computes radial-MLP
    tensor-product messages, scatters them into per-block PSUM via one-hot
    matmuls, then applies linear/skip_tp/residual in a transposed node
    stage.  Host unpermutes and reassembles the full output.
"""

import math
import os
from contextlib import ExitStack

import numpy as np

import concourse.bass as bass
import concourse.tile as tile
import concourse.mybir as mybir
import bass_rust
from concourse import bacc
from concourse.alu_op_type import AluOpType
from concourse.vector_clock import ScopedClock

F32 = mybir.dt.float32
F32R = mybir.dt.float32r
BF16 = mybir.dt.bfloat16
I16 = mybir.dt.int16
AX = bass_rust.AxisListType
AF = mybir.ActivationFunctionType

N_NODES = 10000
MUL = 128
N_SPECIES = 4
N_RADIAL = 8
AVG_NEIGH = 16.0
SQRT3 = 1.7320508075688772
N_CORES = 8
NBLK = 10                # node blocks per core
BINS = N_CORES * NBLK    # 80
NODES_PC = NBLK * 128    # 1280 node slots per core
NODE_PAD = BINS * 128    # 10240

MAX_WAITS = 1


class SplitDrainTileContext(tile.TileContext):
    """This compiler build rejects instructions with >1 sem-wait condition
    ("Too many sync wait commands").  Tile's end-of-kernel drain accumulates
    one wait per live sem; split them across preceding SP nops."""

    def _drain_and_barrier(self, tick_clock, wait_clock):
        nc = self.nc
        probe = nc.sync.nop(nofuse=True, hint="drain_wait_split")
        wait_clock.add_sem_waits(
            probe.ins, ScopedClock({None: tick_clock.global_clock})
        )
        si = probe.ins.sync_info
        waits = list(si.on_wait) if si is not None else []
        if si is not None:
            si.on_wait = waits[:MAX_WAITS]
        for i in range(MAX_WAITS, len(waits), MAX_WAITS):
            n2 = nc.sync.nop(nofuse=True, hint=f"drain_wait_split_{i}")
            n2.ins.sync_info = mybir.SyncInfo(
                on_wait=waits[i : i + MAX_WAITS], on_update=[]
            )
        nc.sync.drain()
        nc.all_engine_barrier()
        assert self.sems is not None
        popped = nc._tile_sem_poison_stack.pop()
        assert popped is self._sem_poison
        nc.clear_and_free_semaphores(list(self.sems.allocated().values()))
        nc.all_engine_barrier()


# ---------------------------------------------------------------- host prep


def _ffd_partition(receiver: np.ndarray):
    """LPT-assign nodes to BINS blocks of <=128 nodes balancing in-degree.
    Returns (S, node_bin, node_row) with every block's edge count <= S."""
    deg = np.bincount(receiver, minlength=N_NODES)
    order = np.argsort(-deg, kind="stable")
    load = np.zeros(BINS, dtype=np.int64)
    cnt = np.zeros(BINS, dtype=np.int64)
    node_bin = np.full(N_NODES, -1, dtype=np.int64)
    node_row = np.full(N_NODES, -1, dtype=np.int64)
    big = np.int64(1) << 60
    for n in order:
        masked = np.where(cnt < 128, load, big)
        b = int(np.argmin(masked))
        assert cnt[b] < 128
        node_bin[n] = b
        node_row[n] = cnt[b]
        load[b] += deg[n]
        cnt[b] += 1
    S = max(128, int(-(-load.max() // 128)) * 128)
    return S, node_bin, node_row


def _host_prep(inputs):
    f32 = np.float32
    node_attrs = np.asarray(inputs["node_attrs"], f32)
    node_feats = np.asarray(inputs["node_feats"], f32)
    edge_attrs = np.asarray(inputs["edge_attrs"], f32)
    edge_feats = np.asarray(inputs["edge_feats"], f32)
    edge_index = np.asarray(inputs["edge_index"])
    sender = edge_index[0].astype(np.int64)
    receiver = edge_index[1].astype(np.int64)
    E = sender.shape[0]

    # residual-mix coefficients (scalar host math on a [1] input)
    ucp = float(np.asarray(inputs["update_coeff_param"]).reshape(-1)[0])
    c = 1.0 / (1.0 + math.exp(-ucp))
    c_old = 1.0 / math.sqrt(c * c + 1.0)
    c_new = c * c_old

    # folded weights
    inv_up = 1.0 / math.sqrt(MUL)
    inv_lin = 1.0 / (math.sqrt(2 * MUL) * AVG_NEIGH)
    inv_sk = 1.0 / math.sqrt(MUL * N_SPECIES)
    Wup0 = (np.asarray(inputs["W_up0"], f32) * inv_up)        # [128,128]
    Wup1 = (np.asarray(inputs["W_up1"], f32) * inv_up)
    W1 = np.asarray(inputs["W_mlp1"], f32) / math.sqrt(N_RADIAL)   # [8,64]
    W2 = np.asarray(inputs["W_mlp2"], f32) / math.sqrt(64.0)       # [64,64]
    W3 = (np.asarray(inputs["W_mlp3"], f32) / math.sqrt(64.0)).copy()  # [64,512]
    W3[:, MUL : 2 * MUL] /= SQRT3                              # fold 1/sqrt(3) into w_b
    # stored as [128, n_blocks*128]: block b cols = rows b*128..(b+1)*128
    Wlin0 = np.asarray(inputs["W_lin0"], f32) * inv_lin        # [256,128]
    Wlin0 = np.concatenate([Wlin0[:128], Wlin0[128:]], axis=1)  # [128,256]
    Wlin1 = np.asarray(inputs["W_lin1"], f32) * inv_lin
    Wlin1 = np.concatenate([Wlin1[:128], Wlin1[128:]], axis=1)
    Wsk0 = (
        np.transpose(np.asarray(inputs["W_skip0"], f32), (1, 0, 2)).reshape(512, 128)
        * (inv_sk * c_new)
    )
    Wsk0 = np.concatenate([Wsk0[v * 128 : (v + 1) * 128] for v in range(4)], axis=1)
    Wsk1 = (
        np.transpose(np.asarray(inputs["W_skip1"], f32), (1, 0, 2)).reshape(512, 128)
        * (inv_sk * c_new)
    )
    Wsk1 = np.concatenate([Wsk1[v * 128 : (v + 1) * 128] for v in range(4)], axis=1)
    identC = (np.eye(128, dtype=f32) * c_old).copy()
    ident = np.eye(128, dtype=f32)
    iota = np.broadcast_to(np.arange(128, dtype=f32), (128, 128)).copy()
    ones1 = np.ones((1, 128), f32)

    # transposed, component-split raw node features [512, NODE_PAD] (bf16)
    import ml_dtypes
    bf16 = ml_dtypes.bfloat16
    nfT = np.zeros((512, NODE_PAD), bf16)
    nfT[0:128, :N_NODES] = node_feats[:, :MUL].T
    v = node_feats[:, MUL:].reshape(N_NODES, MUL, 3)
    nfT[128:256, :N_NODES] = v[:, :, 0].T
    nfT[256:384, :N_NODES] = v[:, :, 1].T
    nfT[384:512, :N_NODES] = v[:, :, 2].T

    # graph partition
    S, node_bin, node_row = _ffd_partition(receiver)
    TPB = S // 128                  # subtiles per block
    EP = NBLK * S                   # padded edges per core
    n_sub = EP // 128               # subtiles per core
    chunks = []                     # (edge_offset, n_edges)
    off = 0
    while off < EP:
        g = min(1024, EP - off)
        chunks.append((off, g))
        off += g
    NCH = len(chunks)

    # per-core node lists
    core_nodes = np.full((N_CORES, NODES_PC), -1, dtype=np.int64)
    slot_of_node = node_bin * 128 + node_row
    for n in range(N_NODES):
        b = node_bin[n]
        core_nodes[b // NBLK, (b % NBLK) * 128 + node_row[n]] = n

    # per-core edge slots
    e_bin = node_bin[receiver]
    e_core = e_bin // NBLK
    e_blk = e_bin % NBLK
    sender16 = np.zeros((N_CORES, EP), np.int16)
    scal = np.zeros((N_CORES, EP, 8), f32)
    scal[:, :, 4] = -1.0
    efTc = np.zeros((N_CORES, 8, EP), f32)
    for k in range(N_CORES):
        for j in range(NBLK):
            es = np.nonzero((e_core == k) & (e_blk == j))[0]
            L = len(es)
            assert L <= S, (k, j, L, S)
            sl = j * S + np.arange(L)
            sender16[k, sl] = sender[es].astype(np.int16)
            scal[k, sl, 0] = edge_attrs[es, 0]
            scal[k][sl, 1:4] = edge_attrs[es, 1:4]
            scal[k, sl, 4] = node_row[receiver[es]].astype(f32)
            efTc[k][:, sl] = edge_feats[es, :].T

    # pack per-chunk SBUF layouts
    scal2 = np.zeros((N_CORES, NCH, 128, 64), f32)
    idx2 = np.zeros((N_CORES, NCH, 128, 64), np.int16)
    for ci, (o, g) in enumerate(chunks):
        t = g // 128
        blk = scal[:, o : o + g, :].reshape(N_CORES, t, 128, 8)
        scal2[:, ci, :, : t * 8] = np.transpose(blk, (0, 2, 1, 3)).reshape(
            N_CORES, 128, t * 8
        )
        w = sender16[:, o : o + g].reshape(N_CORES, g // 16, 16)
        idx2[:, ci, :16, : g // 16] = np.transpose(w, (0, 2, 1))
        for rep in range(1, 8):
            idx2[:, ci, rep * 16 : (rep + 1) * 16, :] = idx2[:, ci, :16, :]

    # flattened to one partition row: base_partition of matmul rhs must be 0
    aT = np.zeros((N_CORES, 1, 4 * NODES_PC), f32)
    for k in range(N_CORES):
        valid = core_nodes[k] >= 0
        a4 = np.zeros((4, NODES_PC), f32)
        a4[:, valid] = node_attrs[core_nodes[k][valid]].T
        aT[k, 0, :] = a4.reshape(-1)

    shared = dict(
        nfT=nfT, Wup0=Wup0.astype(bf16), Wup1=Wup1.astype(bf16),
        W1=W1.astype(bf16), W2=W2.astype(bf16), W3=W3.astype(bf16),
        Wlin0=Wlin0, Wlin1=Wlin1, Wsk0=Wsk0, Wsk1=Wsk1,
        identC=identC, ident=ident, iota=iota, ones1=ones1,
    )
    in_maps = []
    for k in range(N_CORES):
        m = dict(shared)
        m["scal2"] = scal2[k]
        m["idx2"] = idx2[k]
        m["efT"] = np.ascontiguousarray(efTc[k]).astype(bf16)
        m["aT"] = np.ascontiguousarray(aT[k])
        in_maps.append(m)

    return dict(
        S=S, TPB=TPB, EP=EP, NCH=NCH, chunks=chunks, n_sub=n_sub,
        core_nodes=core_nodes, in_maps=in_maps,
    )


# ------------------------------------------------------------- bass program


def _ap(base, col_off, dims):
    """Sub-AP of a tile AP: keep partition dim, replace free dims."""
    return bass.AP(base.tensor, base.offset + col_off, [base.ap[0]] + dims)


def _build_program(prep):
    S = prep["S"]
    TPB = prep["TPB"]
    EP = prep["EP"]
    chunks = prep["chunks"]
    NCH = prep["NCH"]

    nc = bacc.Bacc("TRN2", target_bir_lowering=False, debug=False)

    din = lambda name, shape, dt=F32: nc.dram_tensor(
        name, shape, dt, kind="ExternalInput"
    ).ap()
    nfT = din("nfT", [512, NODE_PAD], BF16)
    Wup0 = din("Wup0", [128, 128], BF16); Wup1 = din("Wup1", [128, 128], BF16)
    W1 = din("W1", [8, 64], BF16); W2 = din("W2", [64, 64], BF16)
    W3 = din("W3", [64, 512], BF16)
    Wlin0 = din("Wlin0", [128, 256], F32R); Wlin1 = din("Wlin1", [128, 256], F32R)
    Wsk0 = din("Wsk0", [128, 512], F32R); Wsk1 = din("Wsk1", [128, 512], F32R)
    identC = din("identC", [128, 128], F32R); ident = din("ident", [128, 128])
    iota = din("iota", [128, 128]); ones1 = din("ones1", [1, 128], F32R)
    scal2 = din("scal2", [NCH, 128, 64])
    idx2 = din("idx2", [NCH, 128, 64], I16)
    efT = din("efT", [8, EP], BF16)
    aT = din("aT", [1, 4 * NODES_PC], F32R)

    table = nc.dram_tensor("table", [NODE_PAD, 512], BF16, kind="Internal").ap()
    outT = nc.dram_tensor("outT", [512, NODES_PC], F32, kind="ExternalOutput").ap()

    r = lambda ap: ap.bitcast(F32R)

    with SplitDrainTileContext(nc) as tc, ExitStack() as octx:
        wpool = octx.enter_context(tc.tile_pool(name="w", bufs=1))

        def wtile(src, shape, dt=F32):
            t = wpool.tile(shape, dt, tag=src.tensor.name)
            nc.sync.dma_start(t[:], src)
            return t

        wup0 = wtile(Wup0, [128, 128], BF16); wup1 = wtile(Wup1, [128, 128], BF16)
        w1 = wtile(W1, [8, 64], BF16); w2 = wtile(W2, [64, 64], BF16)
        w3 = wtile(W3, [64, 512], BF16)
        wlin = [wtile(Wlin0, [128, 256], F32R), wtile(Wlin1, [128, 256], F32R)]
        wsk = [wtile(Wsk0, [128, 512], F32R), wtile(Wsk1, [128, 512], F32R)]
        idc = wtile(identC, [128, 128], F32R); idn = wtile(ident, [128, 128])
        iot = wtile(iota, [128, 128]); on1 = wtile(ones1, [1, 128], F32R)
        at_sb = wtile(aT, [1, 4 * NODES_PC], F32R)
        msvT = octx.enter_context(tc.tile_pool(name="msvT", bufs=1)).tile(
            [128, 8 * NODES_PC], F32R
        )

        # ---------------- stage A: node table = linear_up(node_feats)
        with ExitStack() as actx:
            apool = actx.enter_context(tc.tile_pool(name="stA", bufs=3))
            apsum = actx.enter_context(
                tc.tile_pool(name="stAp", bufs=2, space="PSUM")
            )
            for t in range(NODE_PAD // 128):
                nf4 = apool.tile([128, 512], BF16, tag="nf4")
                src = bass.AP(
                    nfT.tensor,
                    t * 128,
                    [[NODE_PAD, 128], [128 * NODE_PAD, 4], [1, 128]],
                )
                nc.sync.dma_start(nf4[:].rearrange("p (b j) -> p b j", b=4), src)
                ps = apsum.tile([128, 512], F32, tag="psA")
                for b in range(4):
                    w_b = wup0 if b == 0 else wup1
                    nc.tensor.matmul(
                        ps[:, b * 128 : (b + 1) * 128],
                        nf4[:, b * 128 : (b + 1) * 128],
                        w_b[:],
                        start=True,
                        stop=True,
                    )
                ta = apool.tile([128, 512], BF16, tag="tA")
                nc.scalar.copy(ta[:], ps[:])
                nc.sync.dma_start(table[t * 128 : (t + 1) * 128, :], ta[:])

        # ---------------- stage B: edge pipeline
        with ExitStack() as bctx:
            bpool = bctx.enter_context(tc.tile_pool(name="stB", bufs=2))
            spool = bctx.enter_context(tc.tile_pool(name="stBs", bufs=4))
            ph1 = bctx.enter_context(tc.tile_pool(name="ph1", bufs=1, space="PSUM"))
            ph2 = bctx.enter_context(tc.tile_pool(name="ph2", bufs=1, space="PSUM"))
            ptp = bctx.enter_context(tc.tile_pool(name="ptp", bufs=2, space="PSUM"))
            pbl = bctx.enter_context(tc.tile_pool(name="pbl", bufs=1, space="PSUM"))
            ptr = bctx.enter_context(tc.tile_pool(name="ptr", bufs=2, space="PSUM"))

            gs = 0  # global subtile index
            blkp = None  # [lo, hi] psum pair
            for ci, (off, G) in enumerate(chunks):
                T = G // 128
                idx_t = bpool.tile([128, 64], I16, tag="idx")
                nc.sync.dma_start(idx_t[:], idx2[ci])
                sc_t = bpool.tile([128, 64], F32, tag="scal")
                nc.sync.dma_start(sc_t[:], scal2[ci])
                ef_t = bpool.tile([8, 1024], BF16, tag="ef")
                nc.sync.dma_start(ef_t[:, :G], efT[:, off : off + G])
                g_t = bpool.tile([128, 8, 512], BF16, tag="gath")
                nc.gpsimd.dma_gather(
                    out_ap=g_t[:, :T, :],
                    in_ap=table[:],
                    idxs_ap=idx_t[:],
                    num_idxs=G,
                    num_idxs_reg=G,
                    elem_size=512,
                )
                h2list = []
                for q0 in range(0, G, 512):
                    Q = min(512, G - q0)
                    h1p = ph1.tile([64, 512], F32, tag="h1")
                    nc.tensor.matmul(
                        h1p[:, :Q], w1[:], ef_t[:, q0 : q0 + Q],
                        start=True, stop=True,
                    )
                    h1s = spool.tile([64, 512], BF16, tag="h1s")
                    nc.scalar.activation(h1s[:, :Q], h1p[:, :Q], AF.Silu)
                    h2p = ph2.tile([64, 512], F32, tag="h2")
                    nc.tensor.matmul(
                        h2p[:, :Q], w2[:], h1s[:, :Q], start=True, stop=True
                    )
                    h2s = spool.tile([64, 512], BF16, tag="h2s")
                    nc.scalar.activation(h2s[:, :Q], h2p[:, :Q], AF.Silu)
                    h2list.append(h2s)

                for s in range(T):
                    j = gs // TPB      # node block
                    sin = gs % TPB     # subtile within block
                    gs += 1
                    first = sin == 0
                    last = sin == TPB - 1

                    tpw = ptp.tile([128, 512], F32, tag="tpw")
                    h2sub = h2list[s // 4]
                    nc.tensor.matmul(
                        tpw[:], h2sub[:, (s % 4) * 128 : (s % 4 + 1) * 128], w3[:],
                        start=True, stop=True,
                    )
                    tps = spool.tile([128, 512], BF16, tag="tps")
                    nc.scalar.copy(tps[:], tpw[:])

                    # oh4 = [oh*ea_s | oh*eav_x | oh*eav_y | oh*eav_z]
                    rrel = sc_t[:, s * 8 + 4 : s * 8 + 5]
                    oh4 = spool.tile([128, 512], BF16, tag="oh4")
                    nc.vector.scalar_tensor_tensor(
                        oh4[:],
                        _ap(iot[:], 0, [[0, 4], [1, 128]]),
                        rrel,
                        _ap(sc_t[:], s * 8, [[1, 4], [0, 128]]),
                        AluOpType.is_equal,
                        AluOpType.mult,
                    )
                    # R = [A|C|Bx|Dx|By|Dy|Bz|Dz]; A=wa*xs C=wc*xs Bi=wb*xv_i Di=wd*xv_i
                    R = spool.tile([128, 1024], BF16, tag="R")
                    nc.vector.tensor_tensor(
                        R[:, 0:256],
                        _ap(tps[:], 0, [[256, 2], [1, 128]]),
                        _ap(g_t[:], s * 512, [[0, 2], [1, 128]]),
                        AluOpType.mult,
                    )
                    nc.vector.tensor_tensor(
                        R[:, 256:1024],
                        _ap(tps[:], 128, [[0, 3], [256, 2], [1, 128]]),
                        _ap(g_t[:], s * 512 + 128, [[128, 3], [0, 2], [1, 128]]),
                        AluOpType.mult,
                    )
                    if first:
                        blkp = [
                            pbl.tile([128, 512], F32, tag="blkA", name=f"blkA_{j}"),
                            pbl.tile([128, 512], F32, tag="blkB", name=f"blkB_{j}"),
                        ]
                        openers = [None, None]
                    # psum: blkA=[m0|m1|m2x|m2y] blkB=[m2z|m3x|m3y|m3z]
                    oh_s = oh4[:, 0:128]

                    def smm(bank, dst, lhsT, rhs, stop=False):
                        mm = nc.tensor.matmul(
                            dst, lhsT, rhs,
                            start=(openers[bank] is None), stop=stop,
                        )
                        if openers[bank] is None:
                            openers[bank] = mm.ins
                        else:
                            tile.add_dep_helper(
                                mm.ins, openers[bank], sync=False,
                                reason="psum bank zero-order",
                            )
                        return mm

                    smm(0, blkp[0][:, 0:128], oh_s, R[:, 0:128])
                    smm(1, blkp[1][:, 0:128], oh4[:, 384:512], R[:, 128:256])  # m2z
                    smm(
                        1,
                        blkp[1][:, 128:512],
                        oh_s,
                        _ap(R[:], 384, [[256, 3], [1, 128]]),
                        stop=last,
                    )
                    for i3 in range(3):
                        oh_i = oh4[:, (1 + i3) * 128 : (2 + i3) * 128]
                        if i3 < 2:
                            smm(0, blkp[0][:, (2 + i3) * 128 : (3 + i3) * 128],
                                oh_i, R[:, 128:256])
                        smm(
                            0,
                            blkp[0][:, 128:256],
                            oh_i,
                            R[:, (256 + 256 * i3) : (384 + 256 * i3)],
                            stop=(last and i3 == 2),
                        )
                    if last:
                        # msv = [m0|m1|m2x|m2y|m2z|m3x|m3y|m3z]
                        msv = spool.tile([128, 1024], F32, tag="msv")
                        nc.vector.tensor_copy(msv[:, 0:512], blkp[0][:])
                        nc.vector.tensor_copy(msv[:, 512:1024], blkp[1][:])
                        srcs = [
                            msv[:, 0:128],      # m0
                            msv[:, 128:256],    # m1
                            msv[:, 256:384],    # m2x
                            msv[:, 640:768],    # m3x
                            msv[:, 384:512],    # m2y
                            msv[:, 768:896],    # m3y
                            msv[:, 512:640],    # m2z
                            msv[:, 896:1024],   # m3z
                        ]
                        for j8, sap in enumerate(srcs):
                            trp = ptr.tile([128, 128], F32, tag="trp")
                            nc.tensor.transpose(trp[:], sap, idn[:])
                            nc.scalar.copy(
                                msvT[:, j8 * NODES_PC + j * 128 : j8 * NODES_PC + (j + 1) * 128],
                                trp[:],
                            )

        # ---------------- stage C: node stage (transposed layout)
        with ExitStack() as cctx:
            cpool = cctx.enter_context(tc.tile_pool(name="stC", bufs=2))
            arpool = cctx.enter_context(tc.tile_pool(name="stCar", bufs=4))
            pcs = cctx.enter_context(tc.tile_pool(name="pcs", bufs=2, space="PSUM"))
            pco = cctx.enter_context(tc.tile_pool(name="pco", bufs=2, space="PSUM"))
            pca = cctx.enter_context(tc.tile_pool(name="pca", bufs=2, space="PSUM"))

            n0 = 0
            while n0 < NODES_PC:
                W = min(512, NODES_PC - n0)
                arep = []
                for vv in range(4):
                    pa = pca.tile([128, 512], F32, tag="pa")
                    nc.tensor.matmul(
                        pa[:, :W],
                        r(on1[:]),
                        r(at_sb[0:1, vv * NODES_PC + n0 : vv * NODES_PC + n0 + W]),
                        start=True, stop=True,
                    )
                    av = arpool.tile([128, 512], F32, tag=f"arep{vv}")
                    nc.vector.tensor_copy(av[:, :W], pa[:, :W])
                    arep.append(av)
                for comp in range(4):
                    wl = wlin[0] if comp == 0 else wlin[1]
                    wsx = wsk[0] if comp == 0 else wsk[1]
                    psS = pcs.tile([128, 512], F32, tag="psS")
                    for half in range(2):
                        jt = 2 * comp + half
                        nc.tensor.matmul(
                            psS[:, :W],
                            r(wl[:, half * 128 : (half + 1) * 128]),
                            r(msvT[:, jt * NODES_PC + n0 : jt * NODES_PC + n0 + W]),
                            start=(half == 0),
                            stop=(half == 1),
                        )
                    ssb = cpool.tile([128, 512], F32R, tag="ssb")
                    nc.vector.tensor_copy(ssb[:, :W], psS[:, :W])
                    psO = pco.tile([128, 512], F32, tag="psO")
                    nc.tensor.matmul(
                        psO[:, :W], r(idc[:]), r(ssb[:, :W]), start=True, stop=False
                    )
                    for vv in range(4):
                        pv = cpool.tile([128, 512], F32R, tag="pv")
                        nc.vector.tensor_tensor(
                            pv[:, :W], ssb[:, :W], arep[vv][:, :W], AluOpType.mult
                        )
                        nc.tensor.matmul(
                            psO[:, :W],
                            r(wsx[:, vv * 128 : (vv + 1) * 128]),
                            r(pv[:, :W]),
                            start=False,
                            stop=(vv == 3),
                        )
                    osb = cpool.tile([128, 512], F32, tag="osb")
                    nc.vector.tensor_copy(osb[:, :W], psO[:, :W])
                    nc.sync.dma_start(
                        outT[comp * 128 : (comp + 1) * 128, n0 : n0 + W], osb[:, :W]
                    )
                n0 += W

    nc.compile()
    return nc


# ---------------------------------------------------------------- top level


def _unshard(results, prep):
    core_nodes = prep["core_nodes"]
    out = np.zeros((N_NODES, MUL, 4), np.float32)
    for k in range(N_CORES):
        oT = results[k]["outT"]  # [512, 1280]
        valid = core_nodes[k] >= 0
        nodes = core_nodes[k][valid]
        cols = np.nonzero(valid)[0]
        for comp in range(4):
            out[nodes, :, comp] = oT[comp * 128 : (comp + 1) * 128, cols].T
    return out


def kernel(**inputs):
    from concourse import bass_utils

    prep = _host_prep(inputs)
    nc = _build_program(prep)
    trace = bool(int(os.environ.get("KERNEL_TRACE", "0")))
    res = bass_utils.run_bass_kernel_spmd(
        nc,
        prep["in_maps"],
        core_ids=list(range(N_CORES)),
        trace=trace,
    )
    if trace:
        kernel.last_results = res
    return _unshard(res.results, prep)


kernel.last_results = None
